# revision 1
# baseline (speedup 1.0000x reference)
"""Trainium2 Bass kernel for the DGMC-style graph matching module.

Sharding: data-parallel over B x s-half -> 8 cores. Core c owns graph
b = c//2 and s-rows [c*SLOC, (c+1)*SLOC). Edge aggregation (segment_sum)
uses dma_gather + per-chunk 0/1 selector matmuls accumulating in PSUM
(dma_scatter_add races on duplicate indices, so it is not used); each
window's edge list carries 128 self-loops so the psi "x + agg" bias is
part of the same contraction. The pairwise MLP over
D = o_s[:,None]-o_t[None,:] is computed per (4s x TS_t x 32k) tile as:
one K=33 matmul building the broadcast difference (+bm1), a relu, and
one K=128 matmul contracting k with Wm2. Large matmuls run as float32r
(1 cycle/row on the PE vs 4 for plain fp32). r_t = S^T r_s is combined
across cores with a pair AllReduce (the two s-halves of a graph) and a
quad AllGather (across graphs) on compact [*,32] buffers, then one
strided DMA pads rows to 256B for dma_gather. o_s/u_s are precomputed
before the iteration loop (independent of S_hat). bm2 is dropped: a
constant shift of S_hat is invariant under the row softmaxes that
consume it.
"""

import numpy as np

B, CIN, COUT, RIN, ROUT, NPSI = 4, 128, 256, 32, 32, 2
NCORES = 8

_CACHE = {}
_PATCHED = False


def _apply_patches():
    """This neuronxcc build rejects >1 sync-wait per instruction
    ('Too many sync wait commands'). After Tile scheduling, rewrite every
    basic block so an instruction carrying N>1 waits is preceded by N-1
    single-wait NoOps on the same engine (program order on the engine
    sequencer makes this semantically identical)."""
    global _PATCHED
    if _PATCHED:
        return
    _PATCHED = True
    import concourse.tile as tile
    import concourse.mybir as mybir
    from concourse.tile import ScopedClock

    def _split_multi_waits(nc):
        for _key, bbb in list(nc.bb_map.items()):
            bb = bbb.bb if hasattr(bbb, "bb") else bbb
            out = []
            changed = False
            for inst in bb.instructions:
                si = inst.sync_info
                waits = list(si.on_wait) if (si and si.on_wait) else []
                if len(waits) > 1:
                    changed = True
                    for w in waits[:-1]:
                        nop = mybir.InstNoOp(
                            name=f"WSPLIT-{nc.next_id()}",
                            engine=inst.engine,
                            ins=[],
                            outs=[],
                            sync_info=mybir.SyncInfo(on_wait=[w], on_update=[]),
                        )
                        nc.register_instruction(nop, overwrite=True)
                        out.append(nop)
                    si.on_wait = [waits[-1]]
                out.append(inst)
            if changed:
                bb.instructions = out

    def _drain_and_barrier(self, tick_clock, wait_clock):
        nc = self.nc
        drain_inst = nc.sync.drain()
        wait_clock.add_sem_waits(
            drain_inst.ins, ScopedClock({None: tick_clock.global_clock})
        )
        si = drain_inst.ins.sync_info
        waits = list(si.on_wait or [])
        if len(waits) > 1:
            si.on_wait = waits[:1]
            for w in waits[1:]:
                d2 = nc.sync.drain()
                si2 = d2.ins.sync_info
                if si2 is None:
                    d2.ins.sync_info = mybir.SyncInfo(on_wait=[w], on_update=[])
                else:
                    si2.on_wait = [w]
        nc.all_engine_barrier()
        assert self.sems is not None
        popped = nc._tile_sem_poison_stack.pop()
        assert popped is self._sem_poison
        nc.clear_and_free_semaphores(list(self.sems.allocated().values()))
        nc.all_engine_barrier()
        _split_multi_waits(nc)

    tile.TileContext._drain_and_barrier = _drain_and_barrier


def _wrap16(idx):
    return np.ascontiguousarray(idx.reshape(-1, 16).T.astype(np.int16))


def _bucket_edges(src, dst, row0, nwin):
    # Each window also gets 128 self-loop edges (global row -> local row)
    # so the psi "x + agg" bias is folded into the gather+selector matmul.
    out = []
    sl = np.arange(128, dtype=np.int64)
    for w in range(nwin):
        lo = row0 + w * 128
        msk = (dst >= lo) & (dst < lo + 128)
        out.append((np.concatenate([src[msk], lo + sl]),
                    np.concatenate([(dst[msk] - lo).astype(np.int64), sl])))
    return out


def _pad_and_pack(wins, nch):
    nwin = len(wins)
    idx = np.zeros((nwin, 128, nch * 8), np.int16)
    sel = np.zeros((nwin, 128, nch, 128), np.float32)
    for w, (srcs, m) in enumerate(wins):
        k = len(srcs)
        pad = np.zeros(nch * 128, np.int64)
        pad[:k] = srcs
        idx[w] = np.tile(_wrap16(pad), (8, 1))
        pos = np.arange(k)
        sel[w, pos % 128, pos // 128, m] = 1.0
    return idx, sel.reshape(nwin, 128, nch * 128)


def _prep(inputs, N):
    import os
    import ml_dtypes
    _hd = os.environ.get("KHDT", "fp16")
    hnp = {"bf16": ml_dtypes.bfloat16, "fp16": np.float16}.get(_hd, np.float32)
    NB = B * N
    SLOC = NB // NCORES
    SWIN = SLOC // 128
    TWIN = N // 128

    f32 = np.float32
    x_s = np.asarray(inputs["x_s"], f32)
    x_t = np.asarray(inputs["x_t"], f32)
    ei_s = np.asarray(inputs["edge_index_s"], np.int64)
    ei_t = np.asarray(inputs["edge_index_t"], np.int64)
    r = np.asarray(inputs["r"], f32)
    W1 = np.asarray(inputs["W1"], f32)
    b1 = np.asarray(inputs["b1"], f32)
    Wp = np.asarray(inputs["Wp"], f32)
    bp = np.asarray(inputs["bp"], f32)
    Wm1 = np.asarray(inputs["Wm1"], f32)
    bm1 = np.asarray(inputs["bm1"], f32)
    Wm2 = np.asarray(inputs["Wm2"], f32)
    sumw = np.asarray(inputs["sum_weights"], f32)

    r0 = r[0].reshape(NB, RIN)
    r1 = r[1].reshape(NB, RIN)
    xr = np.ascontiguousarray(np.concatenate([x_s, r0, r1], 1))

    swins = [_bucket_edges(ei_s[0], ei_s[1], c * SLOC, SWIN) for c in range(NCORES)]
    twins = [_bucket_edges(ei_t[0], ei_t[1], b * N, TWIN) for b in range(B)]
    nch_s = max(1, max((len(s) + 127) // 128 for ws in swins for (s, _) in ws))
    nch_t = max(1, max((len(s) + 127) // 128 for ws in twins for (s, _) in ws))

    delta = np.tile(np.eye(32, dtype=f32), (1, 4 * 32))  # [32, 4096]
    # w2sel3[ss*32+k, g, 4g+ss] = Wm2[k]: contracts (4s x 32k) hidden rows of
    # group g into the 4 s-rows 4g+ss of a full 128-row PSUM accumulator.
    w2sel = np.zeros((128, 32, 128), f32)
    for g in range(32):
        for ss in range(4):
            w2sel[ss * 32:(ss + 1) * 32, g, 4 * g + ss] = Wm2[:, 0]
    w2sel = w2sel.reshape(128, 32 * 128)
    b1c = np.ascontiguousarray(np.stack([b1[:128], b1[128:]], 1))
    bpc = np.ascontiguousarray(bp.T)
    bm1b = np.tile(bm1[None, :], (128, 1))
    ident = np.eye(128, dtype=f32)
    wp2 = np.ascontiguousarray(Wp.reshape(2 * RIN, ROUT))

    per_core = []
    for c in range(NCORES):
        b = c // 2
        sidx, ssel = _pad_and_pack(swins[c], nch_s)
        tidx, tsel = _pad_and_pack(twins[b], nch_t)
        per_core.append({
            "xr": xr,
            "xtf": x_t,
            "rloc": np.ascontiguousarray(
                np.concatenate([r0[c * SLOC:(c + 1) * SLOC],
                                r1[c * SLOC:(c + 1) * SLOC]], 0)),
            "sidx": sidx, "ssel": ssel,
            "tidx": tidx, "tsel": tsel,
            "w1": W1, "b1c": b1c, "wp2": wp2, "bpc": bpc,
            "wm1": np.ascontiguousarray(Wm1), "bm1b": bm1b,
            "w2sel": w2sel.astype(hnp), "delta": delta.astype(hnp),
            "ident": ident, "ones": np.ones((1, N), hnp),
        })
    sw = [float(sumw[i]) for i in range(NPSI + 1)]
    return per_core, nch_s, nch_t, sw


def _build(N, nch_s, nch_t, sw):
    import os
    PH = int(os.environ.get("KPHASES", "9"))
    _apply_patches()
    import concourse.bacc as bacc
    import concourse.tile as tile
    from concourse import mybir

    NB = B * N
    SLOC = NB // NCORES
    SWIN = SLOC // 128
    TWIN = N // 128
    TS = min(512, N)
    NTW = N // TS
    NSB = SLOC // 128
    NQ = max(1, SLOC // TS)
    DXR = CIN + 2 * RIN
    f32 = mybir.dt.float32
    i16 = mybir.dt.int16
    AF = mybir.ActivationFunctionType
    ALU = mybir.AluOpType
    AX = mybir.AxisListType

    f32r = mybir.dt.float32r
    bf16 = mybir.dt.bfloat16
    fp16 = mybir.dt.float16
    _hd = os.environ.get("KHDT", "fp16")
    hdt = {"bf16": bf16, "fp16": fp16}.get(_hd, f32r)

    def R(ap):
        return ap.bitcast(f32r)

    def H(ap):
        # hidden-MLP DMA sources already carry hdt bits host-side for 16-bit
        # dtypes; for f32r they are plain fp32 bits reinterpreted.
        return ap if hdt != f32r else ap.bitcast(f32r)

    nc = bacc.Bacc(None, target_bir_lowering=False, debug=False,
                   num_devices=NCORES)

    def din(name, shape, dt=f32):
        return nc.dram_tensor(name, list(shape), dt, kind="ExternalInput").ap()

    xr_d = din("xr", (NB, DXR))
    xtf_d = din("xtf", (NB, CIN))
    rloc_d = din("rloc", (2 * SLOC, RIN))
    sidx_d = din("sidx", (SWIN, 128, nch_s * 8), i16)
    ssel_d = din("ssel", (SWIN, 128, nch_s * 128))
    tidx_d = din("tidx", (TWIN, 128, nch_t * 8), i16)
    tsel_d = din("tsel", (TWIN, 128, nch_t * 128))
    w1_d = din("w1", (CIN, COUT))
    b1c_d = din("b1c", (128, 2))
    wp2_d = din("wp2", (2 * RIN, ROUT))
    bpc_d = din("bpc", (32, 2))
    wm1_d = din("wm1", (RIN, ROUT))
    bm1b_d = din("bm1b", (128, 32))
    w2sel_d = din("w2sel", (128, 32 * 128), hdt if hdt != f32r else f32)
    delta_d = din("delta", (32, 4096), hdt if hdt != f32r else f32)
    ident_d = din("ident", (128, 128))
    ones_d = din("ones", (1, N), hdt if hdt != f32r else f32)

    out_d = nc.dram_tensor("out", [SLOC, N], f32, kind="ExternalOutput").ap()

    rtp_d = nc.dram_tensor("rt_partial", [N, RIN], f32).ap()
    rtb_d = nc.dram_tensor("rt_batch", [N, RIN], f32).ap()
    rtg_d = nc.dram_tensor("rt_gath", [NB, RIN], f32).ap()
    rtf_d = nc.dram_tensor("rt_full", [NB, 64], f32).ap()
    adr_d = nc.dram_tensor("a_dram", [2 * SLOC, 32], hdt).ap()

    PAIRS = [[2 * b, 2 * b + 1] for b in range(B)]
    QUADS = [[2 * b + h for b in range(B)] for h in range(2)]

    with tile.TileContext(nc) as tc:
        with (
            tc.tile_pool(name="const", bufs=1) as cpool,
            tc.tile_pool(name="pers", bufs=1) as ppool,
            tc.tile_pool(name="bg", bufs=2) as bgpool,
            tc.tile_pool(name="gath", bufs=2) as gpool,
            tc.tile_pool(name="selp", bufs=2) as spool,
            tc.tile_pool(name="work", bufs=4) as wpool,
            tc.tile_pool(name="psA", bufs=2, space="PSUM") as psA,
            tc.tile_pool(name="psH", bufs=3, space="PSUM") as psH,
            tc.tile_pool(name="psU", bufs=2, space="PSUM") as psU,
        ):
            sync, vec, act, pe, gp = (nc.sync, nc.vector, nc.scalar,
                                      nc.tensor, nc.gpsimd)

            GCH = int(os.environ.get("KGCH", "4"))  # idx per dma_gather /128

            def gather_chunked(gw, src_ap, idxt, nch, elem):
                for q0 in range(0, nch, GCH):
                    q1 = min(q0 + GCH, nch)
                    gp.dma_gather(gw[:, q0:q1, :], src_ap,
                                  idxt[:, q0 * 8:q1 * 8],
                                  num_idxs=(q1 - q0) * 128,
                                  num_idxs_reg=(q1 - q0) * 128,
                                  elem_size=elem)

            # ---- constants ----
            w1t = cpool.tile([CIN, COUT], f32, tag="w1t")
            sync.dma_start(w1t[:], w1_d[:])
            b1t = cpool.tile([128, 2], f32, tag="b1t")
            sync.dma_start(b1t[:], b1c_d[:])
            wpt = [cpool.tile([RIN, ROUT], f32, tag=f"wpt{i}", name=f"wpt{i}")
                   for i in range(2)]
            sync.dma_start(wpt[0][:], wp2_d[0:RIN, :])
            sync.dma_start(wpt[1][:], wp2_d[RIN:2 * RIN, :])
            bpt = cpool.tile([32, 2], f32, tag="bpt")
            sync.dma_start(bpt[:], bpc_d[:])
            wm1t = cpool.tile([RIN, ROUT], f32, tag="wm1t")
            sync.dma_start(wm1t[:], wm1_d[:])
            bm1t = cpool.tile([128, 32], f32, tag="bm1t")
            sync.dma_start(bm1t[:], bm1b_d[:])
            w2t = cpool.tile([128, 32, 128], hdt, tag="w2t")
            sync.dma_start(w2t[:], H(w2sel_d[:].rearrange("p (g m) -> p g m", m=128)))
            idt = cpool.tile([128, 128], f32, tag="idt")
            sync.dma_start(idt[:], ident_d[:])
            rhs33 = cpool.tile([33, N], hdt, tag="rhs33")
            sync.dma_start(rhs33[32:33, :], H(ones_d[:]))
            tidxt = [cpool.tile([128, nch_t * 8], i16, tag=f"tidx{w}",
                                name=f"tidxt{w}") for w in range(TWIN)]
            for w in range(TWIN):
                sync.dma_start(tidxt[w][:], tidx_d[w, :, :])
            zt = cpool.tile([128, 64], f32, tag="zt")
            vec.memset(zt[:], 0.0)
            for k in range(NB // 128):
                sync.dma_start(rtf_d[k * 128:(k + 1) * 128, :], zt[:])

            # ---- persistent ----
            rsT = [ppool.tile([32, SLOC], f32, tag=f"rsT{i}", name=f"rsT{i}")
                   for i in range(2)]
            shat = [ppool.tile([128, N], f32, tag=f"shat{s}", name=f"shat{s}")
                    for s in range(NSB)]
            sacc = [ppool.tile([128, N], f32, tag=f"sacc{s}", name=f"sacc{s}")
                    for s in range(NSB)]
            stil = [ppool.tile([128, N], f32, tag=f"stil{s}", name=f"stil{s}")
                    for s in range(NSB)]
            rtaT = ppool.tile([32, N], f32, tag="rtaT")
            otT = ppool.tile([32, N], f32, tag="otT")

            with tc.tile_pool(name="psi1", bufs=1) as qpool:
                xaggTs = qpool.tile([128, SLOC], f32, tag="xaggTs")
                xaggTt = qpool.tile([128, N], f32, tag="xaggTt")
                hsT = [qpool.tile([128, SLOC], f32, tag=f"hsT{k}",
                                  name=f"hsT{k}") for k in range(2)]
                htT = [qpool.tile([128, N], f32, tag=f"htT{k}",
                                  name=f"htT{k}") for k in range(2)]

                # ---- phase 1: s-side psi1 aggregation ----
                for w in range(SWIN):
                    sit = wpool.tile([128, nch_s * 8], i16, tag="sit")
                    sync.dma_start(sit[:], sidx_d[w, :, :])
                    gw = gpool.tile([128, nch_s, DXR], f32, tag="gw")
                    gather_chunked(gw, xr_d[:], sit[:], nch_s, DXR)
                    sl = spool.tile([128, nch_s * 128], f32, tag="sl")
                    sync.dma_start(sl[:], ssel_d[w, :, :])
                    pagg = psA.tile([128, DXR], f32, tag="pa")
                    for j in range(nch_s):
                        pe.matmul(pagg[:], sl[:, j * 128:(j + 1) * 128],
                                  gw[:, j, :], start=(j == 0),
                                  stop=(j == nch_s - 1))
                    xagg = wpool.tile([128, DXR], f32, tag="xagg")
                    vec.tensor_copy(xagg[:], pagg[:])
                    pt = psA.tile([128, 128], f32, tag="pa")
                    pe.transpose(pt[:], xagg[:, 0:128], idt[:])
                    vec.tensor_copy(xaggTs[:, w * 128:(w + 1) * 128], pt[:])
                    for i in range(2):
                        c0 = CIN + i * RIN
                        ptr = psA.tile([32, 128], f32, tag="pa")
                        pe.transpose(ptr[:], xagg[:, c0:c0 + RIN], idt[:])
                        vec.tensor_copy(rsT[i][:, w * 128:(w + 1) * 128], ptr[:])

                for kh in range(2):
                    for q in range(NQ):
                        q0, q1 = q * TS, min((q + 1) * TS, SLOC)
                        ph = psA.tile([128, TS], f32, tag="pa")
                        pe.matmul(ph[:, 0:q1 - q0],
                                  w1t[:, kh * 128:(kh + 1) * 128],
                                  xaggTs[:, q0:q1], start=True, stop=True)
                        act.activation(hsT[kh][:, q0:q1], ph[:, 0:q1 - q0],
                                       AF.Relu, bias=b1t[:, kh:kh + 1])

                # hoisted: o_s/u_s (independent of S_hat) for both psi
                for i in range(2):
                    osT = wpool.tile([32, SLOC], f32, tag="osT",
                                     name=f"osT{i}")
                    for q in range(NQ):
                        q0, q1 = q * TS, min((q + 1) * TS, SLOC)
                        po = psA.tile([32, TS], f32, tag="pa")
                        pe.matmul(po[:, 0:q1 - q0], wpt[i][:],
                                  rsT[i][:, q0:q1], start=True, stop=True)
                        act.activation(osT[:, q0:q1], po[:, 0:q1 - q0],
                                       AF.Relu, bias=bpt[:, i:i + 1])
                    for sb in range(NSB):
                        pu = psA.tile([128, 32], f32, tag="pa")
                        pe.matmul(pu[:], osT[:, sb * 128:(sb + 1) * 128],
                                  wm1t[:], start=True, stop=True)
                        au = wpool.tile([128, 32], hdt, tag="au")
                        vec.tensor_tensor(au[:], pu[:], bm1t[:], op=ALU.add)
                        sync.dma_start(
                            adr_d[i * SLOC + sb * 128:
                                  i * SLOC + (sb + 1) * 128, :], au[:])

                # ---- phase 2: t-side psi1 aggregation ----
                for w in range(TWIN):
                    gw = gpool.tile([128, nch_t, CIN], f32, tag="gw")
                    gather_chunked(gw, xtf_d[:], tidxt[w][:], nch_t, CIN)
                    sl = spool.tile([128, nch_t * 128], f32, tag="sl")
                    sync.dma_start(sl[:], tsel_d[w, :, :])
                    pagg = psA.tile([128, CIN], f32, tag="pa")
                    for j in range(nch_t):
                        pe.matmul(pagg[:], sl[:, j * 128:(j + 1) * 128],
                                  gw[:, j, :], start=(j == 0),
                                  stop=(j == nch_t - 1))
                    xagg = wpool.tile([128, CIN], f32, tag="xagg")
                    vec.tensor_copy(xagg[:], pagg[:])
                    pt = psA.tile([128, 128], f32, tag="pa")
                    pe.transpose(pt[:], xagg[:], idt[:])
                    vec.tensor_copy(xaggTt[:, w * 128:(w + 1) * 128], pt[:])

                for kh in range(2):
                    for tw in range(NTW):
                        ph = psA.tile([128, TS], f32, tag="pa")
                        pe.matmul(ph[:], w1t[:, kh * 128:(kh + 1) * 128],
                                  xaggTt[:, tw * TS:(tw + 1) * TS],
                                  start=True, stop=True)
                        act.activation(htT[kh][:, tw * TS:(tw + 1) * TS],
                                       ph[:], AF.Relu, bias=b1t[:, kh:kh + 1])

                # ---- phase 3: S_hat ----
                for sb in range(NSB):
                    for tw in range(NTW):
                        ps = psA.tile([128, TS], f32, tag="pa")
                        pe.matmul(ps[:], hsT[0][:, sb * 128:(sb + 1) * 128],
                                  htT[0][:, tw * TS:(tw + 1) * TS],
                                  start=True, stop=False)
                        pe.matmul(ps[:], hsT[1][:, sb * 128:(sb + 1) * 128],
                                  htT[1][:, tw * TS:(tw + 1) * TS],
                                  start=False, stop=True)
                        vec.tensor_copy(shat[sb][:, tw * TS:(tw + 1) * TS], ps[:])

            # ---- iterations ----
            if PH <= 3:
                for sb in range(NSB):
                    vec.tensor_copy(sacc[sb][:], shat[sb][:])
            for i in range(0 if PH > 3 else NPSI + 1, NPSI + 1):
                last = (i == NPSI)
                rinv = []
                for sb in range(NSB):
                    nmx = wpool.tile([128, 1], f32, tag="nmx")
                    vec.tensor_reduce(nmx[:], shat[sb][:], AX.X, ALU.max,
                                      negate=True)
                    rsum = wpool.tile([128, 1], f32, tag="rsum")
                    act.activation(stil[sb][:], shat[sb][:], AF.Exp,
                                   bias=nmx[:], accum_out=rsum[:])
                    ri = wpool.tile([128, 1], f32, tag=f"ri{sb}",
                                    name=f"ri{sb}_{i}")
                    vec.reciprocal(ri[:], rsum[:])
                    rinv.append(ri)

                if not last:
                    # r_t partial = S~^T @ (r_s / rowsum) over local s
                    rss = []
                    for sb in range(NSB):
                        rl = wpool.tile([128, RIN], f32, tag=f"rl{sb}",
                                        name=f"rl{sb}_{i}")
                        sync.dma_start(
                            rl[:], rloc_d[i * SLOC + sb * 128:
                                          i * SLOC + (sb + 1) * 128, :])
                        vec.tensor_scalar_mul(rl[:], rl[:], rinv[sb][:])
                        rss.append(rl)
                    for tt in range(N // 128):
                        prt = psA.tile([128, RIN], f32, tag="pa")
                        for sb in range(NSB):
                            pe.matmul(prt[:],
                                      stil[sb][:, tt * 128:(tt + 1) * 128],
                                      rss[sb][:], start=(sb == 0),
                                      stop=(sb == NSB - 1))
                        rts = wpool.tile([128, RIN], f32, tag="rts")
                        vec.tensor_copy(rts[:], prt[:])
                        sync.dma_start(rtp_d[tt * 128:(tt + 1) * 128, :],
                                       rts[:])

                # fold this softmax into S_acc (in place; after r_t reads)
                for sb in range(NSB):
                    riw = wpool.tile([128, 1], f32, tag="riw")
                    vec.tensor_scalar_mul(riw[:], rinv[sb][:], sw[i])
                    if i == 0:
                        vec.tensor_scalar_mul(sacc[sb][:], stil[sb][:], riw[:])
                    else:
                        vec.tensor_scalar_mul(stil[sb][:], stil[sb][:], riw[:])
                        vec.tensor_tensor(sacc[sb][:], sacc[sb][:], stil[sb][:],
                                          op=ALU.add)
                if last:
                    break

                gp.collective_compute(
                    "AllReduce", ALU.add, replica_groups=PAIRS,
                    ins=[rtp_d[:].opt()], outs=[rtb_d[:].opt()])
                gp.collective_compute(
                    "AllGather", ALU.bypass, replica_groups=QUADS,
                    ins=[rtb_d[:].opt()], outs=[rtg_d[:].opt()])
                # expand compact [NB,32] to the 256B-aligned gather table
                sync.dma_start(rtf_d[:, 0:RIN], rtg_d[:])

                # t-side o_t aggregation from rt_full (self-loops add r_t)
                for w in range(TWIN):
                    gw = gpool.tile([128, nch_t, 64], f32, tag="gw")
                    gather_chunked(gw, rtf_d[:], tidxt[w][:], nch_t, 64)
                    sl = spool.tile([128, nch_t * 128], f32, tag="sl")
                    sync.dma_start(sl[:], tsel_d[w, :, :])
                    pagg = psA.tile([128, 64], f32, tag="pa")
                    for j in range(nch_t):
                        pe.matmul(pagg[:], sl[:, j * 128:(j + 1) * 128],
                                  gw[:, j, :], start=(j == 0),
                                  stop=(j == nch_t - 1))
                    ragg = wpool.tile([128, RIN], f32, tag="ragg")
                    vec.tensor_copy(ragg[:], pagg[:, 0:RIN])
                    ptr = psA.tile([32, 128], f32, tag="pa")
                    pe.transpose(ptr[:], ragg[:], idt[:])
                    vec.tensor_copy(rtaT[:, w * 128:(w + 1) * 128], ptr[:])

                # o_t, u_t -> rhs33 rows 0..31 (negated)
                for tw in range(NTW):
                    po = psA.tile([32, TS], f32, tag="pa")
                    pe.matmul(po[:], wpt[i][:],
                              rtaT[:, tw * TS:(tw + 1) * TS],
                              start=True, stop=True)
                    act.activation(otT[:, tw * TS:(tw + 1) * TS], po[:],
                                   AF.Relu, bias=bpt[:, i:i + 1])
                for tw in range(NTW):
                    pu = psA.tile([32, TS], f32, tag="pa")
                    pe.matmul(pu[:], wm1t[:],
                              otT[:, tw * TS:(tw + 1) * TS],
                              start=True, stop=True)
                    vec.tensor_scalar_mul(rhs33[0:32, tw * TS:(tw + 1) * TS],
                                          pu[:], -1.0)

                # hidden pairwise MLP -> update shat (g-loop software-
                # pipelined depth 2 so mm2(g) never stalls the PE on relu(g))
                for sb in range(NSB if PH >= 5 else 0):
                    bigt = bgpool.tile([33, 4096], hdt, tag="bigt")
                    sync.dma_start(bigt[0:32, :], H(delta_d[:]))
                    sync.dma_start(
                        bigt[32:33, :],
                        adr_d[i * SLOC + sb * 128:i * SLOC + (sb + 1) * 128, :]
                        .rearrange("a b -> (a b)")[None, :])
                    for tw in range(NTW):
                        pup = psU.tile([128, TS], f32, tag="pu")
                        shs = {}
                        for g in range(34):
                            if g < 32:
                                phh = psH.tile([128, TS], f32, tag="ph")
                                pe.matmul(phh[:],
                                          bigt[:, g * 128:(g + 1) * 128],
                                          rhs33[:, tw * TS:(tw + 1) * TS],
                                          start=True, stop=True)
                                sh = wpool.tile([128, TS], hdt, tag="sh")
                                if g % 2 == 0:
                                    act.activation(sh[:], phh[:], AF.Relu)
                                else:
                                    vec.tensor_scalar_max(sh[:], phh[:], 0.0)
                                shs[g] = sh
                            if g >= 2:
                                gg = g - 2
                                pe.matmul(pup[:], w2t[:, gg, :], shs.pop(gg)[:],
                                          start=(gg == 0), stop=(gg == 31))
                        vec.tensor_tensor(shat[sb][:, tw * TS:(tw + 1) * TS],
                                          shat[sb][:, tw * TS:(tw + 1) * TS],
                                          pup[:], op=ALU.add)

            # ---- final softmax of S_acc ----
            for sb in range(NSB):
                nmx = wpool.tile([128, 1], f32, tag="nmx")
                vec.tensor_reduce(nmx[:], sacc[sb][:], AX.X, ALU.max,
                                  negate=True)
                rsum = wpool.tile([128, 1], f32, tag="rsum")
                act.activation(stil[sb][:], sacc[sb][:], AF.Exp, bias=nmx[:],
                               accum_out=rsum[:])
                ri = wpool.tile([128, 1], f32, tag="rif")
                vec.reciprocal(ri[:], rsum[:])
                vec.tensor_scalar_mul(stil[sb][:], stil[sb][:], ri[:])
                sync.dma_start(out_d[sb * 128:(sb + 1) * 128, :], stil[sb][:])

    nc.compile()
    return nc


def _get_nc(inputs, N):
    per_core, nch_s, nch_t, sw = _prep(inputs, N)
    import os
    key = (N, nch_s, nch_t, tuple(sw), os.environ.get("KPHASES", "9"),
           os.environ.get("KHDT", "fp16"), os.environ.get("KGCH", "4"))
    if key not in _CACHE:
        _CACHE[key] = _build(N, nch_s, nch_t, sw)
    return _CACHE[key], per_core


def kernel(**inputs) -> np.ndarray:
    from concourse.bass_utils import run_bass_kernel_spmd

    N = np.asarray(inputs["x_s"]).shape[0] // B
    nc, per_core = _get_nc(inputs, N)
    res = run_bass_kernel_spmd(nc, per_core, list(range(NCORES))).results
    out = np.concatenate([res[c]["out"] for c in range(NCORES)], 0)
    return out.astype(np.float32)


def timed_run(inputs):
    """Run with NTFF profiling; returns (out, exec_time_ns)."""
    import os
    import shutil
    from concourse.bass_utils import run_bass_kernel_spmd

    N = np.asarray(inputs["x_s"]).shape[0] // B
    nc, per_core = _get_nc(inputs, N)
    tdir = os.environ.get("KTRACE_DIR")
    if tdir:
        shutil.rmtree(tdir, ignore_errors=True)
        os.makedirs(tdir, exist_ok=True)
    r = run_bass_kernel_spmd(nc, per_core, list(range(NCORES)), trace=True,
                             tmpdir=tdir)
    out = np.concatenate([r.results[c]["out"] for c in range(NCORES)], 0)
    t = r.exec_time_ns
    if r.mean_exec_time_ns:
        t = r.mean_exec_time_ns
    return out.astype(np.float32), t

